# revision 27
# baseline (speedup 1.0000x reference)
"""Trainium2 Bass kernel for nn_Attention (dense transformer block-attention).

Reference semantics (faithful reshape WITHOUT head transpose):
  qkv = x @ w_qkv                    # [B, N, 3*1024]
  q = qkv[..., 0:1024].reshape(B, 16, 2048, 64)   # head h <- token rows [h*128,(h+1)*128)
  out[b, n, c] = O_head(n//128)[(n%128)*16 + c//64, c%64]

Sharding: 32 (b, head) pairs over 8 cores -> each core: 1 batch x 4 heads.
Pure data parallel, no collectives. Host preps xT (bf16) per core + full w (bf16).

Layout tricks:
- Sub-token permutation n2' = cb*128 + r (softmax is permutation-invariant
  over keys; queries un-permuted via the output index mapping).
- qT/kT hold the 64-wide head dim DUPLICATED on both partition halves, so
  S matmuls contract K=128 (computing 2*q.k; factor folded into exp scale)
  and the layout transposes are clean [128,128] PE transposes.
- PV: out^T = [v|ones].T @ exp(S^T): softmax denominators ride in row 64.
- One PSUM layout all kernel long: tag ps = 2x[128,1024] (4 banks) used by
  projection accumulators / S ping-pong / tail transposes, tag po =
  1x[65,2048] (4 banks) for PV accumulators. No phase barriers.
"""

import numpy as np
import ml_dtypes

B, N, D = 2, 2048, 1024
H_PER_CORE = 4          # head-blocks per core
ROWS = 128              # token rows per head-block
SUB = 2048              # sub-tokens per head (128 rows * 16 col-blocks)
DH = 64                 # head dim
CB = 16                 # col-blocks per row
SCALE = 0.125           # 64 ** -0.5
N_CORES = 8

_GRAPH = None


def build_graph():
    global _GRAPH
    if _GRAPH is not None:
        return _GRAPH

    import concourse.mybir as mybir
    import concourse.tile as tile
    from concourse import bacc
    from concourse.masks import make_identity
    from contextlib import ExitStack

    f32 = mybir.dt.float32
    bf16 = mybir.dt.bfloat16
    EXP = mybir.ActivationFunctionType.Exp

    nc = bacc.Bacc("TRN2", target_bir_lowering=False, debug=False,
                   num_devices=N_CORES)

    xt_dram = nc.dram_tensor("xt", [D, H_PER_CORE * ROWS], bf16,
                             kind="ExternalInput")
    w_dram = nc.dram_tensor("w", [D, 3 * D], bf16, kind="ExternalInput")
    out_dram = nc.dram_tensor("out", [H_PER_CORE * ROWS, D], f32,
                              kind="ExternalOutput")

    KO = D // 128  # 8 k-tiles

    with tile.TileContext(nc) as tc, ExitStack() as ctx:
        const_pool = ctx.enter_context(tc.tile_pool(name="const", bufs=1))
        in_pool = ctx.enter_context(tc.tile_pool(name="inputs", bufs=1))
        qk_pool = ctx.enter_context(tc.tile_pool(name="qk", bufs=4))
        head_pool = ctx.enter_context(tc.tile_pool(name="head", bufs=1))
        pt_pool = ctx.enter_context(tc.tile_pool(name="pt", bufs=4))
        ot_pool = ctx.enter_context(tc.tile_pool(name="ot", bufs=3))
        small_pool = ctx.enter_context(tc.tile_pool(name="small", bufs=16))
        trt_pool = ctx.enter_context(tc.tile_pool(name="trt", bufs=16))
        psum = ctx.enter_context(tc.tile_pool(name="psum", bufs=2,
                                              space="PSUM"))
        psum2 = ctx.enter_context(tc.tile_pool(name="psum2", bufs=2,
                                               space="PSUM"))
        opsum = ctx.enter_context(tc.tile_pool(name="opsum", bufs=1,
                                               space="PSUM"))

        # ---- constants ----
        ident = const_pool.tile([128, 128], f32, tag="ident")
        make_identity(nc, ident[:])
        ident_bf = const_pool.tile([128, 128], bf16, tag="ident_bf")
        make_identity(nc, ident_bf[:])
        # warm up the exp table while the projection runs
        warm = const_pool.tile([128, 1], f32, tag="warm")
        nc.vector.memset(warm[:], 0.0)
        nc.scalar.activation(warm[:], warm[:], EXP)

        # ---- input DMA in first-consumption order ----
        xt_sbuf = in_pool.tile([128, KO, H_PER_CORE * ROWS], bf16, tag="xt")
        w_sbuf = in_pool.tile([128, KO, 3 * D], bf16, tag="w")
        nc.sync.dma_start(xt_sbuf[:, 0, 0:ROWS],
                          xt_dram.ap()[0:128, 0:ROWS])
        nc.sync.dma_start(w_sbuf[:, 0, 0:512], w_dram.ap()[0:128, 0:512])
        nc.sync.dma_start(xt_sbuf[:, 0, ROWS:],
                          xt_dram.ap()[0:128, ROWS:])
        nc.sync.dma_start(w_sbuf[:, 0, 512:1024],
                          w_dram.ap()[0:128, 512:1024])
        for ko in range(1, KO):
            nc.sync.dma_start(xt_sbuf[:, ko, :],
                              xt_dram.ap()[ko * 128:(ko + 1) * 128, :])
            nc.sync.dma_start(
                w_sbuf[:, ko, 0:1024],
                w_dram.ap()[ko * 128:(ko + 1) * 128, 0:1024])
        for ko in range(KO):
            nc.sync.dma_start(
                w_sbuf[:, ko, 1024:2048],
                w_dram.ap()[ko * 128:(ko + 1) * 128, 1024:2048])
        for cols in ((2048, 2560), (2560, 3072)):
            for ko in range(KO):
                nc.sync.dma_start(
                    w_sbuf[:, ko, cols[0]:cols[1]],
                    w_dram.ap()[ko * 128:(ko + 1) * 128,
                                cols[0]:cols[1]])

        # persistent per-head tiles (qT/kT carry duplicated d-halves).
        # Head 0 has its own tiles (filled via PE transposes); heads 1-3
        # live in combined [128, 3, SUB] tiles filled by the transposed
        # projection (one strided copy covers all three heads).
        qT0 = head_pool.tile([128, SUB], bf16, tag="qT0", name="qT0")
        kT0 = head_pool.tile([128, SUB], bf16, tag="kT0", name="kT0")
        qTg = head_pool.tile([128, 3, SUB], bf16, tag="qTg", name="qTg")
        kTg = head_pool.tile([128, 3, SUB], bf16, tag="kTg", name="kTg")

        def qT_ap(t, lo, hi):
            return qT0[:, lo:hi] if t == 0 else qTg[:, t - 1, lo:hi]

        def kT_ap(t, lo, hi):
            return kT0[:, lo:hi] if t == 0 else kTg[:, t - 1, lo:hi]
        # v_ones padded to 128 stationary columns: full-width LDWEIGHTS
        # stays on the fast background path instead of serializing ~54ns
        # before every PV matmul. Pad cols are zeroed once.
        v_ones = [head_pool.tile([128, CB, 128], bf16, tag=f"vo{t}",
                                 name=f"vo{t}")
                  for t in range(H_PER_CORE)]
        for t in range(H_PER_CORE):
            nc.vector.memset(v_ones[t][:, :, DH], 1.0)
            nc.vector.memset(v_ones[t][:, :, DH + 1:], 0.0)

        # ---- phase 1: projection (per block) ----
        qk2s = [None] * H_PER_CORE

        def emit_proj(t, ncxs=range(6)):
            if qk2s[t] is None:
                qk2s[t] = qk_pool.tile([128, 2 * CB, 128], bf16, tag="qk2",
                                       name=f"qk2_{t}")
            qk2 = qk2s[t]
            # [128,512] accumulators from the small psum pool so the
            # attention pipeline (ps ping-pong + po) is never contended
            for ncx in ncxs:
                ps = psum2.tile([128, 512], f32, tag="ps2")
                for ko in range(KO):
                    nc.tensor.matmul(
                        ps[:],
                        xt_sbuf[:, ko, t * ROWS:(t + 1) * ROWS],
                        w_sbuf[:, ko, ncx * 512:(ncx + 1) * 512],
                        start=(ko == 0), stop=(ko == KO - 1))
                src = ps[:].rearrange("p (a b) -> p a b", b=DH)
                if ncx < 4:
                    nc.vector.tensor_copy(
                        qk2[:, ncx * 8:(ncx + 1) * 8, 0:DH], src)
                    nc.vector.tensor_copy(
                        qk2[:, ncx * 8:(ncx + 1) * 8, DH:128], src)
                else:
                    nc.vector.tensor_copy(
                        v_ones[t][:, (ncx - 4) * 8:(ncx - 3) * 8, 0:DH],
                        src)

        def emit_transposes(t):
            # head 0 gates the first attention: PE transposes (fast)
            for cb in range(2 * CB):
                pst = psum2.tile([128, 128], bf16, tag="ps2")
                nc.tensor.transpose(pst[:], qk2s[t][:, cb, :],
                                    ident_bf[:])
                c = cb % CB
                dst = (qT_ap(t, c * 128, (c + 1) * 128) if cb < CB
                       else kT_ap(t, c * 128, (c + 1) * 128))
                nc.vector.tensor_copy(dst, pst[:])

        def emit_one_dt(dt):
            # Heads 1-3 q/k without PE transposes: accumulate
            # w_tile^T @ xt over ko so psum lands directly in
            # [qkv dims, 384 token rows] layout covering all three heads.
            # Two [64, 3, 128] cast-copies evacuate the cb sub-blocks;
            # an SBUF->SBUF DMA then duplicates the d-halves partition-
            # wise (DVE lanes cannot cross partitions).
            if True:
                ps = psum2.tile([128, 512], f32, tag="ps2")
                for ko in range(KO):
                    nc.tensor.matmul(
                        ps[:, 0:384],
                        w_sbuf[:, ko, dt * 128:(dt + 1) * 128],
                        xt_sbuf[:, ko, ROWS:],
                        start=(ko == 0), stop=(ko == KO - 1))
                tgt = qTg if dt < 8 else kTg
                cbase = (dt % 8) * 2
                psv = ps[:, 0:384].rearrange("p (h r) -> p h r", r=ROWS)
                for c2 in range(2):
                    cb = cbase + c2
                    nc.vector.tensor_copy(
                        tgt[0:DH, :, cb * 128:(cb + 1) * 128],
                        psv[c2 * DH:(c2 + 1) * DH, :, :])
                nc.sync.dma_start(
                    tgt[DH:128, :, cbase * 128:(cbase + 2) * 128],
                    tgt[0:DH, :, cbase * 128:(cbase + 2) * 128])

        def emit_pass(t, ihalf, OTt, fill_each_j=None):
            po = opsum.tile([128, SUB // 2], f32, tag="po")
            for j in range(CB):
                ps = psum.tile([128, 1024], f32, tag="ps")
                for sub in range(2):
                    ic = ihalf * 2 + sub
                    nc.tensor.matmul(
                        ps[:, sub * 512:(sub + 1) * 512],
                        kT_ap(t, j * 128, (j + 1) * 128),
                        qT_ap(t, ic * 512, (ic + 1) * 512),
                        start=True, stop=True)
                pt = pt_pool.tile([128, 1024], bf16, tag="pt")
                # psum holds 2*(q.k) (duplicated halves) -> scale/2
                nc.scalar.activation(pt[:], ps[:], EXP, scale=SCALE / 2)
                for sub in range(2):
                    nc.tensor.matmul(
                        po[:, sub * 512:(sub + 1) * 512],
                        v_ones[t][:, j, :],
                        pt[:, sub * 512:(sub + 1) * 512],
                        start=(j == 0), stop=(j == CB - 1))
                if fill_each_j is not None:
                    fill_each_j(j)
            # evacuate the half-accumulator on DVE (idle during attention;
            # first in its FIFO region so the po slot frees promptly)
            nc.vector.tensor_copy(
                OTt[0:DH + 1, ihalf * 1024:(ihalf + 1) * 1024],
                po[0:DH + 1, :])

        def emit_tail_half(t, ihalf, OTt):
            # PE transpose (psum2) -> normalize (DVE) -> out DMA.  The
            # final head's outs go on the (empty) Sync HWDGE queue so the
            # kernel end isn't paced by GpSimd's slow SWDGE issue.
            out_eng = nc.sync if t == H_PER_CORE - 1 else nc.gpsimd
            for cb in range(ihalf * 8, ihalf * 8 + 8):
                ptr = psum2.tile([128, DH + 1], bf16, tag="ps2")
                nc.tensor.transpose(
                    ptr[:],
                    OTt[0:DH + 1, cb * 128:(cb + 1) * 128],
                    ident_bf[0:DH + 1, 0:DH + 1])
                recip = small_pool.tile([128, 1], f32, tag="recip")
                nc.vector.reciprocal(recip[:], ptr[:, DH:DH + 1])
                outt = small_pool.tile([128, DH], f32, tag="outt")
                nc.vector.tensor_scalar_mul(outt[:], ptr[:, 0:DH],
                                            recip[:])
                out_eng.dma_start(
                    out_dram.ap()[t * ROWS:(t + 1) * ROWS,
                                  cb * DH:(cb + 1) * DH],
                    outt[:])

        # ---- program order: later heads' projection/transposes deferred
        # so they fill PE gaps inside earlier ACT-bound attention ----
        OTs = [ot_pool.tile([DH + 1, SUB], bf16, tag="OTf", name=f"OTf{t}")
               for t in range(H_PER_CORE)]
        # head 0: q,k projection + transposes first (gated on 4MB of w),
        # v projection deferred — its weight cols are the last 2MB of the
        # DMA stream; PV j consumes v_ones ~1.1µs/j so it can lag the
        # first S matmuls by several µs without stalling the pipeline.
        emit_proj(0, ncxs=range(4))
        emit_transposes(0)
        # dt block BEFORE pass(0,0): it is ready at ~26µs (q,k weights +
        # xt only) and outranks the S stream, so all projection work
        # finishes up front — the attention window then holds only
        # S/PV + v-projs (~126µs PE) under 137.6µs of ACT work and runs
        # ACT-paced instead of stealing 15µs+ of mid-stream stalls.
        # Emitted before head-0's v so its psum2 rotation isn't chained
        # behind v-column-gated accumulators.
        for _dt in range(16):
            emit_one_dt(_dt)
        emit_proj(0, ncxs=range(4, 6))
        emit_pass(0, 0, OTs[0])
        emit_proj(1, ncxs=range(4, 6))
        emit_pass(0, 1, OTs[0])
        emit_tail_half(0, 0, OTs[0])
        emit_proj(2, ncxs=range(4, 6))
        emit_pass(1, 0, OTs[1])
        emit_tail_half(0, 1, OTs[0])
        emit_proj(3, ncxs=range(4, 6))
        emit_pass(1, 1, OTs[1])
        emit_tail_half(1, 0, OTs[1])
        emit_pass(2, 0, OTs[2])
        emit_tail_half(1, 1, OTs[1])
        emit_pass(2, 1, OTs[2])
        emit_tail_half(2, 0, OTs[2])
        emit_pass(3, 0, OTs[3])
        emit_tail_half(2, 1, OTs[2])
        emit_pass(3, 1, OTs[3])
        emit_tail_half(3, 0, OTs[3])
        emit_tail_half(3, 1, OTs[3])

    nc.compile()
    _GRAPH = nc
    return nc


def make_in_maps(x, w_qkv):
    w_bf = np.ascontiguousarray(w_qkv).astype(ml_dtypes.bfloat16)
    maps = []
    for c in range(N_CORES):
        b = c // 4
        r0 = (c % 4) * H_PER_CORE * ROWS
        xt = np.ascontiguousarray(
            x[b, r0:r0 + H_PER_CORE * ROWS, :].T).astype(ml_dtypes.bfloat16)
        maps.append({"xt": xt, "w": w_bf})
    return maps


def assemble_out(results):
    out = np.empty((B, N, D), dtype=np.float32)
    for c in range(N_CORES):
        b = c // 4
        r0 = (c % 4) * H_PER_CORE * ROWS
        out[b, r0:r0 + H_PER_CORE * ROWS, :] = results[c]["out"]
    return out


def kernel(x, w_qkv):
    from concourse import bass_utils
    nc = build_graph()
    res = bass_utils.run_bass_kernel_spmd(
        nc, make_in_maps(np.asarray(x), np.asarray(w_qkv)),
        list(range(N_CORES)))
    return assemble_out(res.results)

